# revision 1
# baseline (speedup 1.0000x reference)
"""Trainium2 Bass kernel for a dense transformer encoder block (B=4, S=2048,
D=1024, H=16, MLP=4096).

Sharding: 8 cores = 4 batch elements x 2 query-halves, no collectives. Each
core's kv sequence is host-reordered so its 1024 query tokens come first
(attention is permutation-invariant over keys), so Q/residual tensors are
plain slices of the kv set. K/V are computed for the full 2048-token sequence
(~6% duplicated FLOPs vs. perfect sharding).

Per-core dataflow is feature-major ("T" = [feature, token]) so every matmul
has contraction dim 128 on partitions (sub-128-contraction matmuls fail to
load on this stack, all dtypes):
  LN1 (token-major, bn_stats) -> PE-transpose -> xnT            [phase 1]
  per head-group of 4 heads: Q/K/V projections from xnT         [phase 3]
    scores^T = KT_pair^T @ Qpad   (zero-padded rhs selects one head
                                   of the packed pair; K=128 kept)
    exp on ACT, scale=1/8 fused, both heads in one [128,1024] op -> f32r
    AV+den fused: lhsT = [V_head | 1 | 0] so psum rows 0:64 = V^T e and
      row 64 = sum(e); one augmented matmul per (ktile, head)
    reciprocal of row 64, partition-broadcast via DRAM round-trip DMA
      (stride-0 partition APs are DRAM-only), multiply -> RT; head B's
      rows shift 0:64 -> 64:128 via a small SBUF->SBUF DMA
  O-proj +bo, PE-transpose back, +residual -> x2 -> DRAM        [phase 4a]
  LN2 on x2 -> PE-transpose -> xn2T                             [phase 4b]
  MLP: h1 (+b1 and exact-erf Gelu fused on ACT), h2 (+b2),      [phase 5]
    PE-transpose back, +x2 residual -> out

Numerics: matmuls in float32r (TF32-class, ~1.5e-4 rel err, full PE rate at
free-dim >= 256; requires producers typed f32r), fp32 PSUM accumulation,
fp32 layernorm/softmax scalars. End-to-end rel err ~1.4e-4.

Weights are host-retiled to [tile, partition, kd, m] so each weight-tile DMA
is one contiguous block (4KB per-partition chunks). LN affine (g=1, b=0 for
this problem's inputs) is skipped at build time when the host detects
identity values; a full-affine variant is built otherwise.

Cost-model (TimelineSim) span: ~862 us/core; PE busy ~820 us (the binding
engine; attention runs at 50% array utilization, the price of the K=128
constraint with DH=64 heads and no working sub-128 row/col tiling).
"""

import os
import sys

sys.path.insert(0, "/opt/trn_rl_repo")

from contextlib import ExitStack

import numpy as np

import concourse.bass as bass
import concourse.tile as tile
from concourse import bacc, bass_utils, mybir
from concourse.masks import make_identity

F32 = mybir.dt.float32
F32R = mybir.dt.float32r
BF16 = mybir.dt.bfloat16
AF = mybir.ActivationFunctionType
ALU = mybir.AluOpType

B, S, D = 4, 2048, 1024
H, DH, MLP = 16, 64, 4096
P = 128
KD = D // P            # 8 partition tiles over D
FT = MLP // P          # 32 partition tiles over MLP dim
NQ = S // 2            # 1024 query tokens per core
ST = S // P            # 16 kv token tiles
QTT = NQ // P          # 8 q token tiles
QS = 512               # free-dim slice
NQS = NQ // QS         # 2
NKS = S // QS          # 4
NG = 4                 # head groups
EPS = 1e-6
DEBUG = bool(int(os.environ.get("KERNEL_DEBUG", "0")))
MLP_BF16 = bool(int(os.environ.get("KERNEL_MLP_BF16", "0")))

_CACHE = {}


def _build(ln_affine=True, mlp_bf16=True):
    nc = bacc.Bacc(None, target_bir_lowering=False, debug=False, num_devices=8)

    xkv = nc.dram_tensor("xkv", [S, D], F32, kind="ExternalInput").ap()
    # weights arrive host-tiled: [tile, p, kd, m] so each SBUF weight tile is
    # one contiguous DRAM block (4KB+ per-partition DMA chunks)
    Wq = nc.dram_tensor("Wq", [KD, P, KD, P], F32R, kind="ExternalInput").ap()
    Wk = nc.dram_tensor("Wk", [KD, P, KD, P], F32R, kind="ExternalInput").ap()
    Wv = nc.dram_tensor("Wv", [NG, P, KD, 256], F32R, kind="ExternalInput").ap()
    Wo = nc.dram_tensor("Wo", [KD, P, KD, P], F32R, kind="ExternalInput").ap()
    W1 = nc.dram_tensor("W1", [FT, P, KD, P], F32R, kind="ExternalInput").ap()
    W2 = nc.dram_tensor("W2", [KD, P, FT, P], BF16 if mlp_bf16 else F32R, kind="ExternalInput").ap()
    bq = nc.dram_tensor("bq", [D], F32, kind="ExternalInput").ap()
    bk = nc.dram_tensor("bk", [D], F32, kind="ExternalInput").ap()
    bv = nc.dram_tensor("bv", [D], F32, kind="ExternalInput").ap()
    bo = nc.dram_tensor("bo", [D], F32, kind="ExternalInput").ap()
    b1 = nc.dram_tensor("b1", [MLP], F32, kind="ExternalInput").ap()
    b2 = nc.dram_tensor("b2", [D], F32, kind="ExternalInput").ap()
    g1 = nc.dram_tensor("g1", [D], F32, kind="ExternalInput").ap()
    be1 = nc.dram_tensor("be1", [D], F32, kind="ExternalInput").ap()
    g2 = nc.dram_tensor("g2", [D], F32, kind="ExternalInput").ap()
    be2 = nc.dram_tensor("be2", [D], F32, kind="ExternalInput").ap()
    out = nc.dram_tensor("out", [NQ, D], F32, kind="ExternalOutput").ap()

    dbg = {}
    if DEBUG:
        dbg["xnkvT"] = nc.dram_tensor("d_xnkvT", [P, KD, S], F32R, kind="ExternalOutput").ap()
        dbg["qt0"] = nc.dram_tensor("d_qt0", [P, 2, NQ], F32R, kind="ExternalOutput").ap()
        dbg["kt0"] = nc.dram_tensor("d_kt0", [P, 2, S], F32R, kind="ExternalOutput").ap()
        dbg["v0"] = nc.dram_tensor("d_v0", [P, ST, 2, 2, P], F32R, kind="ExternalOutput").ap()
        dbg["rt"] = nc.dram_tensor("d_rt", [P, KD, NQ], F32R, kind="ExternalOutput").ap()
        dbg["e0"] = nc.dram_tensor("d_e0", [P, QS], F32R, kind="ExternalOutput").ap()
        dbg["s0"] = nc.dram_tensor("d_s0", [P, QS], F32, kind="ExternalOutput").ap()
        dbg["av0"] = nc.dram_tensor("d_av0", [65, 2, QS], F32, kind="ExternalOutput").ap()
        dbg["x2"] = nc.dram_tensor("d_x2", [P, QTT, D], F32, kind="ExternalOutput").ap()

    def bcast_ap(vec):
        # [D] dram vector -> [128, D] partition-replicated DMA source
        return bass.AP(tensor=vec.tensor, offset=vec.offset, ap=[[0, P]] + list(vec.ap))



    with tile.TileContext(nc) as tc:
        es = ExitStack()
        params = es.enter_context(tc.tile_pool(name="params", bufs=1))
        dramp = es.enter_context(tc.tile_pool(name="dram", bufs=1, space="DRAM"))
        x2d = dramp.tile([P, QTT, D], F32)

        ident_f = params.tile([P, P], F32)
        make_identity(nc, ident_f)
        ident = params.tile([P, P], F32R)
        nc.vector.tensor_copy(ident[:], ident_f[:])
        ones_f = params.tile([P, 1], F32)
        nc.vector.memset(ones_f[:, 0:1], 1.0)

        def pvec(v, n, nm):  # [n*128] -> [128, n] (dim o*128+p -> [p, o])
            t = params.tile([P, n], F32, name=nm)
            nc.sync.dma_start(t[:], v.rearrange("(o p) -> p o", p=P))
            return t

        bq_t = pvec(bq, KD, "bq_t")
        bk_t = pvec(bk, KD, "bk_t")
        bo_t = pvec(bo, KD, "bo_t")
        b2_t = pvec(b2, KD, "b2_t")
        b1_t = pvec(b1, FT, "b1_t")
        bv_rep = params.tile([P, D], F32)
        nc.gpsimd.dma_start(bv_rep[:], bcast_ap(bv))

        rt_es = ExitStack()
        rtp = rt_es.enter_context(tc.tile_pool(name="rt", bufs=1))
        RT_h = [rtp.tile([P, KD, QS], F32R, name=f"RT{h}") for h in range(NQS)]

        xn_es = ExitStack()
        xnp = xn_es.enter_context(tc.tile_pool(name="xn", bufs=1))
        xn_kvT = xnp.tile([P, KD, S], F32R)

        # ---- Phase 1: LN1 + transpose to feature-major ----
        with tc.tile_pool(name="p1tmp", bufs=4) as p1t, \
             tc.tile_pool(name="p1s", bufs=4) as p1s, \
             tc.tile_pool(name="ln1", bufs=1) as ln1p, \
             tc.tile_pool(name="p1ps", bufs=6, space="PSUM") as ps1:
            g1_rep = ln1p.tile([P, D], F32)
            nc.gpsimd.dma_start(g1_rep[:], bcast_ap(g1))
            be1_rep = ln1p.tile([P, D], F32)
            nc.gpsimd.dma_start(be1_rep[:], bcast_ap(be1))
            eps_t = ln1p.tile([P, 1], F32)
            nc.vector.memset(eps_t[:], EPS)

            for t in range(ST):
                x_t = p1t.tile([P, D], F32, tag="x_t")
                nc.sync.dma_start(x_t[:], xkv[t * P:(t + 1) * P, :])
                stats = p1s.tile([P, 2, 6], F32, tag="stats")
                xv = x_t[:].rearrange("p (s f) -> p s f", s=2)
                for s in range(2):
                    nc.vector.bn_stats(stats[:, s, :], xv[:, s, :])
                mv = p1s.tile([P, 2], F32, tag="mv")
                nc.vector.bn_aggr(mv[:], stats[:])
                std = p1s.tile([P, 1], F32, tag="std")
                nc.scalar.activation(std[:], mv[:, 1:2], AF.Sqrt, bias=eps_t[:])
                nc.vector.reciprocal(std[:], std[:])
                xn_t = p1t.tile([P, D], F32R, tag="xn_t")
                nc.vector.tensor_scalar(
                    xn_t[:], x_t[:], scalar1=mv[:, 0:1], scalar2=std[:],
                    op0=ALU.subtract, op1=ALU.mult)
                if ln_affine:
                    nc.vector.tensor_tensor(xn_t[:], xn_t[:], g1_rep[:], ALU.mult)
                    nc.vector.tensor_tensor(xn_t[:], xn_t[:], be1_rep[:], ALU.add)
                for j2 in range(KD // 2):
                    pst = ps1.tile([P, 2, P], F32, tag="tp")
                    for h in range(2):
                        nc.tensor.transpose(
                            pst[:, h, :].bitcast(F32R),
                            xn_t[:, (2 * j2 + h) * P:(2 * j2 + h + 1) * P], ident[:])
                    nc.vector.tensor_copy(
                        xn_kvT[:, 2 * j2:2 * j2 + 2, t * P:(t + 1) * P], pst[:])

        if DEBUG:
            nc.sync.dma_start(dbg["xnkvT"], xn_kvT[:])

        # ---- Phase 3: per-group QKV projection + attention ----
        with tc.tile_pool(name="kv", bufs=1) as kvp, \
             tc.tile_pool(name="wst", bufs=2) as wsp, \
             tc.tile_pool(name="expp", bufs=2) as expp, \
             tc.tile_pool(name="qpad", bufs=1) as qpp, \
             tc.tile_pool(name="rcbc", bufs=1) as rcp, \
             tc.tile_pool(name="aps", bufs=1, space="PSUM") as aps:

            zsc = qpp.tile([P, QS], F32)
            nc.vector.memset(zsc[:], 0.0)
            qpadA = [qpp.tile([P, QS], F32R, name=f"qpadA{i}") for i in range(1)]
            qpadB = [qpp.tile([P, QS], F32R, name=f"qpadB{i}") for i in range(1)]
            for i in range(1):
                nc.vector.tensor_copy(qpadA[i][:], zsc[:])
                nc.vector.tensor_copy(qpadB[i][:], zsc[:])

            QT_g = kvp.tile([P, 2, NQ], F32R)
            KT_g = kvp.tile([P, 2, S], F32R)
            # per (toktile, pair, head j): [V_head(64) | 1 | 0(63)]
            V_gp = kvp.tile([P, ST, 2, 2, P], F32R)
            for t in range(ST):
                nc.vector.tensor_copy(
                    V_gp[:, t], zsc[:].rearrange("p (a b m) -> p a b m", a=2, b=2))
            one_r = qpp.tile([P, 1], F32R)
            nc.vector.tensor_copy(one_r[:], ones_f[:, 0:1])
            for t in range(ST):
                for pi in range(2):
                    for j in range(2):
                        nc.vector.tensor_copy(V_gp[:, t, pi, j, 64:65], one_r[:])
            it_count = 0

            for g in range(NG):
                for pl in range(2):   # head pairs 2g, 2g+1
                    pr = 2 * g + pl
                    wq_t = wsp.tile([P, KD, P], F32R, tag="wq_t")
                    nc.sync.dma_start(wq_t[:], Wq[pr])
                    for q in range(NQS):
                        ps = aps.tile([P, QS], F32, tag="pp", bufs=2)
                        for kd in range(KD):
                            nc.tensor.matmul(
                                ps[:], wq_t[:, kd, :], xn_kvT[:, kd, q * QS:(q + 1) * QS],
                                start=(kd == 0), stop=(kd == KD - 1))
                        nc.vector.tensor_scalar_add(
                            QT_g[:, pl, q * QS:(q + 1) * QS], ps[:], bq_t[:, pr:pr + 1])
                    wk_t = wsp.tile([P, KD, P], F32R, tag="wk_t")
                    nc.sync.dma_start(wk_t[:], Wk[pr])
                    for q in range(NKS):
                        ps = aps.tile([P, QS], F32, tag="pp", bufs=2)
                        for kd in range(KD):
                            nc.tensor.matmul(
                                ps[:], wk_t[:, kd, :], xn_kvT[:, kd, q * QS:(q + 1) * QS],
                                start=(kd == 0), stop=(kd == KD - 1))
                        nc.vector.tensor_scalar_add(
                            KT_g[:, pl, q * QS:(q + 1) * QS], ps[:], bk_t[:, pr:pr + 1])
                wv_t = wsp.tile([P, KD, 256], F32R, tag="wv_t", bufs=1)
                nc.sync.dma_start(wv_t[:], Wv[g])
                for t in range(ST):
                    ps = aps.tile([P, QS], F32, tag="pp", bufs=2)
                    for kd in range(KD):
                        nc.tensor.matmul(
                            ps[:, 0:256], xn_kvT[:, kd, t * P:(t + 1) * P], wv_t[:, kd, :],
                            start=(kd == 0), stop=(kd == KD - 1))
                    for pi in range(2):
                        nc.vector.tensor_tensor(
                            V_gp[:, t, pi, :, 0:64],
                            ps[:, pi * 128:(pi + 1) * 128].rearrange("p (j m) -> p j m", j=2),
                            bv_rep[:, g * 256 + pi * 128:g * 256 + (pi + 1) * 128].rearrange(
                                "p (j m) -> p j m", j=2), ALU.add)

                if DEBUG and g == 0:
                    nc.sync.dma_start(dbg["kt0"], KT_g[:])
                    nc.sync.dma_start(dbg["v0"], V_gp[:])
                    nc.sync.dma_start(dbg["qt0"], QT_g[:])

                for q in range(NQS):
                    for pl in range(2):
                        pr = 2 * g + pl
                        i = it_count % 1
                        it_count += 1
                        qsl = slice(q * QS, (q + 1) * QS)
                        nc.vector.tensor_copy(qpadA[i][0:64, :], QT_g[0:64, pl, qsl])
                        nc.vector.tensor_copy(qpadB[i][64:128, :], QT_g[64:128, pl, qsl])
                        av1 = aps.tile([P, QS], F32, tag="av1")
                        av2 = aps.tile([P, QS], F32, tag="av2")
                        for kt in range(ST):
                            ktsl = slice(kt * P, (kt + 1) * P)
                            sAB = aps.tile([P, 2, QS], F32, tag="sAB", bufs=2)
                            nc.tensor.matmul(sAB[:, 0, :], KT_g[:, pl, ktsl], qpadA[i][:],
                                             start=True, stop=True)
                            nc.tensor.matmul(sAB[:, 1, :], KT_g[:, pl, ktsl], qpadB[i][:],
                                             start=True, stop=True)
                            eAB = expp.tile([P, 2, QS], F32R, tag="eAB")
                            nc.scalar.activation(eAB[:], sAB[:], AF.Exp, scale=0.125)
                            eA = eAB[:, 0, :]
                            eB = eAB[:, 1, :]
                            if DEBUG and g == 0 and q == 0 and pl == 0 and kt == 0:
                                nc.sync.dma_start(dbg["e0"], eA)
                                s0c = rcp.tile([P, QS], F32, tag="s0c")
                                nc.vector.tensor_copy(s0c[:], sAB[:, 0, :])
                                nc.sync.dma_start(dbg["s0"], s0c[:])
                            st, sp = (kt == 0), (kt == ST - 1)
                            nc.tensor.matmul(av1[:], V_gp[:, kt, pl, 0, :], eA,
                                             start=st, stop=sp, skip_group_check=True)
                            nc.tensor.matmul(av2[:], V_gp[:, kt, pl, 1, :], eB,
                                             start=st, stop=sp, skip_group_check=True)
                        # free the av psums fast: copy to SBUF, divide from there
                        avc = rcp.tile([65, 2, QS], F32, tag="avc")
                        nc.vector.tensor_copy(avc[0:65, 0, :], av1[0:65, :])
                        nc.vector.tensor_copy(avc[0:65, 1, :], av2[0:65, :])
                        nc.vector.reciprocal(avc[64:65, 0, :], avc[64:65, 0, :])
                        nc.vector.reciprocal(avc[64:65, 1, :], avc[64:65, 1, :])
                        rcd = dramp.tile([2, QS], F32, tag="rcd", bufs=2)
                        nc.sync.dma_start(rcd[0:1, :], avc[64:65, 0, :])
                        nc.sync.dma_start(rcd[1:2, :], avc[64:65, 1, :])
                        bcA = rcp.tile([64, QS], F32, tag="bcA")
                        bcB = rcp.tile([64, QS], F32, tag="bcB")

                        def _b64(row_ap):
                            return bass.AP(tensor=row_ap.tensor, offset=row_ap.offset,
                                           ap=[[0, 64]] + list(row_ap.ap)[1:])

                        nc.sync.dma_start(bcA[:], _b64(rcd[0:1, :]))
                        nc.sync.dma_start(bcB[:], _b64(rcd[1:2, :]))
                        if DEBUG and g == 0 and q == 0 and pl == 0:
                            nc.sync.dma_start(dbg["av0"], avc[:])
                        nc.vector.tensor_tensor(RT_h[q][0:64, pr, :], avc[0:64, 0, :], bcA[:], ALU.mult)
                        stB = rcp.tile([64, QS], F32R, tag="stB")
                        nc.vector.tensor_tensor(stB[:], avc[0:64, 1, :], bcB[:], ALU.mult)
                        nc.sync.dma_start(RT_h[q][64:128, pr, :], stB[:])

        xn_es.close()

        if DEBUG:
            for h in range(NQS):
                nc.sync.dma_start(
                    dbg["rt"].rearrange("p k (h w) -> p k h w", h=NQS)[:, :, h, :], RT_h[h][:])

        # ---- Phase 4a: O-projection + residual -> x2 (DRAM) ----
        with tc.tile_pool(name="p4tmp", bufs=2) as p4t, \
             tc.tile_pool(name="p4ps", bufs=2, space="PSUM") as ps4, \
             tc.tile_pool(name="p4tps", bufs=6, space="PSUM") as ps4t:
            for q in range(NQS):
                attnT = p4t.tile([P, KD, QS], F32R, tag="attnT")
                for mt in range(KD):
                    wo_t = p4t.tile([P, KD, P], F32R, tag="wo_t")
                    nc.sync.dma_start(wo_t[:], Wo[mt])
                    ps = ps4.tile([P, QS], F32, tag="pp")
                    for kd in range(KD):
                        nc.tensor.matmul(
                            ps[:], wo_t[:, kd, :], RT_h[q][:, kd, :],
                            start=(kd == 0), stop=(kd == KD - 1))
                    nc.vector.tensor_scalar_add(
                        attnT[:, mt, :], ps[:], bo_t[:, mt:mt + 1])
                for j in range(QS // P):
                    tt = q * (QS // P) + j
                    xr_t = p4t.tile([P, D], F32, tag="xr_t")
                    nc.sync.dma_start(xr_t[:], xkv[tt * P:(tt + 1) * P, :])
                    x2_t = p4t.tile([P, D], F32, tag="x2_t")
                    for m2 in range(KD // 2):
                        pst = ps4t.tile([P, 2, P], F32, tag="tp")
                        for h in range(2):
                            nc.tensor.transpose(
                                pst[:, h, :].bitcast(F32R),
                                attnT[:, 2 * m2 + h, j * P:(j + 1) * P], ident[:])
                        nc.vector.tensor_tensor(
                            x2_t[:, 2 * m2 * P:(2 * m2 + 2) * P],
                            pst[:].rearrange("p a m -> p (a m)"),
                            xr_t[:, 2 * m2 * P:(2 * m2 + 2) * P], ALU.add)
                    nc.sync.dma_start(x2d[:, tt, :], x2_t[:])
                    if DEBUG:
                        nc.sync.dma_start(dbg["x2"][:, tt, :], x2_t[:])
        rt_es.close()

        # ---- Phase 4b: LN2 -> xn2T ----
        xn2_es = ExitStack()
        xn2p = xn2_es.enter_context(tc.tile_pool(name="xn2", bufs=1))
        xn2T_h = [xn2p.tile([P, KD, QS], F32R, name=f"xn2T{h}") for h in range(NQS)]
        with tc.tile_pool(name="p4btmp", bufs=4) as p4bt, \
             tc.tile_pool(name="p4bs", bufs=4) as p4bs, \
             tc.tile_pool(name="ln2", bufs=1) as ln2p, \
             tc.tile_pool(name="p4bps", bufs=6, space="PSUM") as ps4b:
            g2_rep = ln2p.tile([P, D], F32)
            nc.gpsimd.dma_start(g2_rep[:], bcast_ap(g2))
            be2_rep = ln2p.tile([P, D], F32)
            nc.gpsimd.dma_start(be2_rep[:], bcast_ap(be2))
            eps2_t = ln2p.tile([P, 1], F32)
            nc.vector.memset(eps2_t[:], EPS)

            for tt in range(QTT):
                x2_t = p4bt.tile([P, D], F32, tag="x2_t")
                nc.sync.dma_start(x2_t[:], x2d[:, tt, :])
                stats = p4bs.tile([P, 2, 6], F32, tag="stats2")
                xv = x2_t[:].rearrange("p (s f) -> p s f", s=2)
                for s in range(2):
                    nc.vector.bn_stats(stats[:, s, :], xv[:, s, :])
                mv = p4bs.tile([P, 2], F32, tag="mv2")
                nc.vector.bn_aggr(mv[:], stats[:])
                std = p4bs.tile([P, 1], F32, tag="std2")
                nc.scalar.activation(std[:], mv[:, 1:2], AF.Sqrt, bias=eps2_t[:])
                nc.vector.reciprocal(std[:], std[:])
                xn2_t = p4bt.tile([P, D], F32R, tag="xn2_t")
                nc.vector.tensor_scalar(
                    xn2_t[:], x2_t[:], scalar1=mv[:, 0:1], scalar2=std[:],
                    op0=ALU.subtract, op1=ALU.mult)
                if ln_affine:
                    nc.vector.tensor_tensor(xn2_t[:], xn2_t[:], g2_rep[:], ALU.mult)
                    nc.vector.tensor_tensor(xn2_t[:], xn2_t[:], be2_rep[:], ALU.add)
                hs_i, loc = tt // (QS // P), (tt % (QS // P)) * P
                for j2 in range(KD // 2):
                    pst = ps4b.tile([P, 2, P], F32, tag="tp")
                    for h in range(2):
                        nc.tensor.transpose(
                            pst[:, h, :].bitcast(F32R),
                            xn2_t[:, (2 * j2 + h) * P:(2 * j2 + h + 1) * P], ident[:])
                    nc.vector.tensor_copy(
                        xn2T_h[hs_i][:, 2 * j2:2 * j2 + 2, loc:loc + P], pst[:])

        # ---- Phase 5: MLP (h1 in bf16, single full-width token pass) ----
        with tc.tile_pool(name="p5tmp", bufs=3) as p5t, \
             tc.tile_pool(name="h1", bufs=1) as h1p, \
             tc.tile_pool(name="w2st", bufs=2) as w2p, \
             tc.tile_pool(name="p5ps", bufs=2, space="PSUM") as ps5, \
             tc.tile_pool(name="p5tps", bufs=4, space="PSUM") as ps5t:
            mdt = BF16 if mlp_bf16 else F32R
            n_hslice = 1 if mlp_bf16 else NQS
            HW_ = NQ // n_hslice
            out_acc = h1p.tile([P, QTT, D], F32)
            for hs in range(n_hslice):
                h1T = h1p.tile([P, FT, HW_], mdt, tag="h1T")
                for ft in range(FT):
                    w1_t = p5t.tile([P, KD, P], F32R, tag="w1_t")
                    nc.sync.dma_start(w1_t[:], W1[ft])
                    for sl in range(HW_ // QS):
                        gsl = (hs * HW_ + sl * QS) // QS
                        ps = ps5.tile([P, QS], F32, tag="pp")
                        for kd in range(KD):
                            nc.tensor.matmul(
                                ps[:], w1_t[:, kd, :], xn2T_h[gsl][:, kd, :],
                                start=(kd == 0), stop=(kd == KD - 1))
                        nc.scalar.activation(h1T[:, ft, sl * QS:(sl + 1) * QS], ps[:],
                                             AF.Gelu, bias=b1_t[:, ft:ft + 1])
                for mt in range(KD):
                    w2_t = w2p.tile([P, FT, P], mdt, tag="w2_t")
                    nc.sync.dma_start(w2_t[:], W2[mt])
                    for sl in range(HW_ // QS):
                        ssl_loc = slice(sl * QS, (sl + 1) * QS)
                        ps = ps5.tile([P, QS], F32, tag="pp")
                        for ft in range(FT):
                            nc.tensor.matmul(
                                ps[:], w2_t[:, ft, :], h1T[:, ft, ssl_loc],
                                start=(ft == 0), stop=(ft == FT - 1))
                        outT = p5t.tile([P, QS], F32R, tag="outT", bufs=2)
                        nc.vector.tensor_scalar_add(outT[:], ps[:], b2_t[:, mt:mt + 1])
                        for j in range(QS // P):
                            tt = hs * (HW_ // P) + sl * (QS // P) + j
                            pst = ps5t.tile([P, P], F32, tag="tp")
                            nc.tensor.transpose(pst[:].bitcast(F32R),
                                                outT[:, j * P:(j + 1) * P], ident[:])
                            nc.vector.tensor_copy(out_acc[:, tt, mt * P:(mt + 1) * P], pst[:])
            for tt in range(QTT):
                x2_t = p5t.tile([P, D], F32, tag="x2r_t")
                nc.sync.dma_start(x2_t[:], x2d[:, tt, :])
                ob = p5t.tile([P, D], F32, tag="ob")
                nc.vector.tensor_tensor(ob[:], out_acc[:, tt, :], x2_t[:], ALU.add)
                nc.sync.dma_start(out[tt * P:(tt + 1) * P, :], ob[:])

        xn2_es.close()
        es.close()

    nc.compile()
    return nc


def kernel(**inputs):
    inputs = {k: np.ascontiguousarray(np.asarray(v), dtype=np.float32)
              for k, v in inputs.items()}
    ln_affine = not (
        np.all(inputs["ln1_g"] == 1.0) and np.all(inputs["ln1_b"] == 0.0)
        and np.all(inputs["ln2_g"] == 1.0) and np.all(inputs["ln2_b"] == 0.0))
    key = ("nc", ln_affine, MLP_BF16)
    if key not in _CACHE:
        _CACHE[key] = _build(ln_affine=ln_affine, mlp_bf16=MLP_BF16)
    nc = _CACHE[key]

    x = inputs["x"]
    def tile_w(W, n_out, m):
        # [Din, Dout] -> [Dout/m, 128, Din/128, m]
        Din, Dout = W.shape
        return np.ascontiguousarray(
            W.reshape(Din // P, P, n_out, m).transpose(2, 1, 0, 3))

    shared = {
        "Wq": tile_w(inputs["Wq"], KD, P), "Wk": tile_w(inputs["Wk"], KD, P),
        "Wv": tile_w(inputs["Wv"], NG, 256), "Wo": tile_w(inputs["Wo"], KD, P),
        "W1": tile_w(inputs["W1"], FT, P),
        "W2": (tile_w(inputs["W2"], KD, P).astype(__import__("ml_dtypes").bfloat16)
               if MLP_BF16 else tile_w(inputs["W2"], KD, P)),
        "bq": inputs["bq"], "bk": inputs["bk"], "bv": inputs["bv"], "bo": inputs["bo"],
        "b1": inputs["b1"], "b2": inputs["b2"],
        "g1": inputs["ln1_g"], "be1": inputs["ln1_b"],
        "g2": inputs["ln2_g"], "be2": inputs["ln2_b"],
    }
    in_maps = []
    for c in range(8):
        b, half = c // 2, c % 2
        m = dict(shared)
        # query half first; attention is permutation-invariant over kv order
        m["xkv"] = np.ascontiguousarray(
            np.concatenate([x[b, half * NQ:(half + 1) * NQ, :],
                            x[b, (1 - half) * NQ:(2 - half) * NQ, :]], axis=0))
        in_maps.append(m)

    trace = bool(int(os.environ.get("KERNEL_TRACE", "0")))
    kw = {}
    if trace:
        kw = dict(trace=True, tmpdir=os.environ.get("KERNEL_TRACE_DIR") or None)
    res = bass_utils.run_bass_kernel_spmd(nc, in_maps, core_ids=list(range(8)), **kw)
    _CACHE["last_results"] = res
    _CACHE["nc"] = nc
    _CACHE["last_in_maps"] = in_maps

    outa = np.empty((B, S, D), dtype=np.float32)
    for c in range(8):
        b, half = c // 2, c % 2
        outa[b, half * NQ:(half + 1) * NQ, :] = res.results[c]["out"]
    return outa



# revision 32
# speedup vs baseline: 1.3768x; 1.3768x over previous
"""Trainium2 Bass kernel for a dense transformer encoder block (B=4, S=2048,
D=1024, H=16, MLP=4096), fp8 DoubleRow edition.

Sharding: 8 cores = 4 batch x 2 query-halves, no collectives. Each core's kv
sequence is host-reordered so its 1024 query tokens come first; K/V are
computed for the full 2048 tokens.

All heavy matmuls run as fp8e4m3 DoubleRow (0.5 cyc/row, 256-deep
contraction). Weights are host-scaled by powers of two (x16 qkvo, x32 W1,
x64 W2) to center fp8 range; every rescale folds into an existing epilogue
or activation scale, so it costs nothing and is numerically exact.

  scores: lhsT = stride-0 j-duplicated K-pair tile, rhs = stride-0
    j-duplicated half-zeroed Q plane -> psum = 2*(16Q.16K) = 512*QK;
    exp(psum/4096 - 3) on ACT -> e fp8  (the -3 shift guards fp8 overflow
    and cancels exactly in softmax).
  AV: V stored in contiguous 256B slots [V(64) | 1/16 | zeros(63)] per
    (ktile-pair, head); one DoubleRow matmul per pair accumulates V^T e in
    psum rows 0:64 and den/16 in row 64. recip -> 16/den, DRAM-roundtrip
    partition broadcast, multiply -> RT = 256*r in fp8.
  O/h1/h2 similarly; MLP precision mode selectable (KERNEL_MLP env):
    f8 (default, all fp8-DR), h1bf (xn2/W1 bf16), bf (whole MLP bf16).

LN + softmax statistics in fp32. Residual adds in fp32.
"""

import os
import sys

sys.path.insert(0, "/opt/trn_rl_repo")

from contextlib import ExitStack

import numpy as np

import concourse.bass as bass
import concourse.tile as tile
from concourse import bacc, bass_utils, mybir
from concourse.masks import make_identity

F32 = mybir.dt.float32
BF16 = mybir.dt.bfloat16
F8 = mybir.dt.float8e4
AF = mybir.ActivationFunctionType
ALU = mybir.AluOpType
PM = mybir.MatmulPerfMode

B, S, D = 4, 2048, 1024
H, DH, MLP = 16, 64, 4096
P = 128
KD = D // P           # 8 feature tiles over D
NQ = S // 2           # 1024 own query tokens
ST = S // P           # 16 kv token tiles
TP = ST // 2          # 8 kv ktile pairs
QTT = NQ // P         # 8 own token tiles
QS = 512
NQS = NQ // QS        # 2
NKS = S // QS         # 4
NPR = H // 2          # 8 head pairs
FT = MLP // P         # 32
EPS = 1e-6
MLP_MODE = os.environ.get("KERNEL_MLP", "bf")   # f8 | h1bf | bf

_CACHE = {}


def _dup2(ap):
    """[128, X] AP -> [128, 2(stride 0), X]: duplicates the plane for
    DoubleRow; the second pass contributes an identical term (result x2)."""
    return bass.AP(tensor=ap.tensor, offset=ap.offset,
                   ap=[list(ap.ap[0]), [0, 2]] + [list(d) for d in ap.ap[1:]])


def _bcast_ap(vec, n):
    return bass.AP(tensor=vec.tensor, offset=vec.offset,
                   ap=[[0, n]] + list(vec.ap))


def _build(affine=False, bv_nz=False, mlp_mode="f8"):
    nc = bacc.Bacc(None, target_bir_lowering=False, debug=False, num_devices=8)

    MD = F8 if mlp_mode == "f8" else BF16          # xn2T / W1 dtype
    HD = BF16 if mlp_mode == "bf" else F8          # h1T / W2 dtype

    xkv = nc.dram_tensor("xkv", [S, D], F32, kind="ExternalInput").ap()
    # stationary weight tiles, contiguous [p, j, m] per (dout-tile, k-pair)
    Wq = nc.dram_tensor("Wq", [KD, P, 4, 2, P], F8, kind="ExternalInput").ap()
    Wk = nc.dram_tensor("Wk", [KD, P, 4, 2, P], F8, kind="ExternalInput").ap()
    Wo = nc.dram_tensor("Wo", [KD, P, 4, 2, P], F8, kind="ExternalInput").ap()
    # Wv is the moving operand of the V projection: [p, kpair, j, dout]
    Wv = nc.dram_tensor("Wv", [P, KD, D], F8, kind="ExternalInput").ap()
    if mlp_mode == "f8":
        W1 = nc.dram_tensor("W1", [FT, P, 4, 2, P], F8, kind="ExternalInput").ap()
    else:
        W1 = nc.dram_tensor("W1", [FT, P, KD, P], BF16, kind="ExternalInput").ap()
    if mlp_mode == "bf":
        W2 = nc.dram_tensor("W2", [KD, P, FT, P], BF16, kind="ExternalInput").ap()
    else:
        W2 = nc.dram_tensor("W2", [KD, P, 16, 2, P], F8, kind="ExternalInput").ap()
    bq = nc.dram_tensor("bq", [D], F32, kind="ExternalInput").ap()    # x16
    bk = nc.dram_tensor("bk", [D], F32, kind="ExternalInput").ap()    # x16
    bv = nc.dram_tensor("bv", [D], F32, kind="ExternalInput").ap()    # x16
    bo = nc.dram_tensor("bo", [D], F32, kind="ExternalInput").ap()
    b1 = nc.dram_tensor("b1", [MLP], F32, kind="ExternalInput").ap()
    b2 = nc.dram_tensor("b2", [D], F32, kind="ExternalInput").ap()
    g1 = nc.dram_tensor("g1", [D], F32, kind="ExternalInput").ap()
    be1 = nc.dram_tensor("be1", [D], F32, kind="ExternalInput").ap()
    g2 = nc.dram_tensor("g2", [D], F32, kind="ExternalInput").ap()
    be2 = nc.dram_tensor("be2", [D], F32, kind="ExternalInput").ap()
    out = nc.dram_tensor("out", [NQ, D], F32, kind="ExternalOutput").ap()

    with tile.TileContext(nc) as tc:
        es = ExitStack()
        params = es.enter_context(tc.tile_pool(name="params", bufs=1))
        dramp = es.enter_context(tc.tile_pool(name="dram", bufs=1, space="DRAM"))

        identf = params.tile([P, P], F32)
        make_identity(nc, identf)
        ident = params.tile([P, P], BF16)
        nc.vector.tensor_copy(ident[:], identf[:])

        def pvec(v, n, nm):  # [n*128] dram -> [128, n] sbuf (p-major)
            t = params.tile([P, n], F32, name=nm)
            nc.sync.dma_start(t[:], v.rearrange("(o p) -> p o", p=P))
            return t

        bq_t = pvec(bq, KD, "bq_t")
        bk_t = pvec(bk, KD, "bk_t")
        bo_t = pvec(bo, KD, "bo_t")
        b2_t = pvec(b2, KD, "b2_t")
        b1_t = pvec(b1, FT, "b1_t")
        if bv_nz:
            bv_rep = params.tile([P, D], F32)
            nc.gpsimd.dma_start(bv_rep[:], _bcast_ap(bv, P))
        if affine:
            g1_rep = params.tile([P, D], F32)
            nc.gpsimd.dma_start(g1_rep[:], _bcast_ap(g1, P))
            be1_rep = params.tile([P, D], F32)
            nc.gpsimd.dma_start(be1_rep[:], _bcast_ap(be1, P))
            g2_rep = params.tile([P, D], F32)
            nc.gpsimd.dma_start(g2_rep[:], _bcast_ap(g2, P))
            be2_rep = params.tile([P, D], F32)
            nc.gpsimd.dma_start(be2_rep[:], _bcast_ap(be2, P))
        eps_t = params.tile([P, 1], F32)
        nc.vector.memset(eps_t[:], EPS)
        neg3_t = params.tile([P, 1], F32)
        nc.vector.memset(neg3_t[:], -3.0)
        maskA = params.tile([P, 1], F32)
        nc.vector.memset(maskA[0:64, :], 1.0)
        nc.vector.memset(maskA[64:128, :], 0.0)
        maskB = params.tile([P, 1], F32)
        nc.vector.memset(maskB[0:64, :], 0.0)
        nc.vector.memset(maskB[64:128, :], 1.0)
        bqA = params.tile([P, KD], F32)
        nc.vector.tensor_copy(bqA[:], bq_t[:])
        nc.vector.memset(bqA[64:128, :], 0.0)
        bqB = params.tile([P, KD], F32)
        nc.vector.tensor_copy(bqB[:], bq_t[:])
        nc.vector.memset(bqB[0:64, :], 0.0)

        # ---------------- persistent activations ----------------
        # created longest-lived first (LIFO pool closes); xn innermost so it
        # can be freed mid-driver.
        x2_es = ExitStack()
        x2p = x2_es.enter_context(tc.tile_pool(name="x2", bufs=1))
        x2 = x2p.tile([P, QTT, D], F32)

        rt_es = ExitStack()
        rtp = rt_es.enter_context(tc.tile_pool(name="rt", bufs=1))
        RT = rtp.tile([P, NPR, NQ], F8)             # 256*r, packed pairs

        w_es = ExitStack()
        p2w = w_es.enter_context(tc.tile_pool(name="p2w", bufs=2))
        epool = w_es.enter_context(tc.tile_pool(name="ep", bufs=2))
        avpool = w_es.enter_context(tc.tile_pool(name="avp", bufs=2))
        p4t = w_es.enter_context(tc.tile_pool(name="p4t", bufs=2))
        p4s = w_es.enter_context(tc.tile_pool(name="p4s", bufs=4))
        attp = w_es.enter_context(tc.tile_pool(name="attp", bufs=1))
        ps_es = ExitStack()
        prjps = ps_es.enter_context(tc.tile_pool(name="prjps", bufs=2, space="PSUM"))
        tpps = ps_es.enter_context(tc.tile_pool(name="tpps", bufs=1, space="PSUM"))

        kv_es = ExitStack()
        kvp = kv_es.enter_context(tc.tile_pool(name="kv", bufs=1))
        Qpad = kvp.tile([P, H, NQ], F8)             # per-head half-zeroed
        KT = kvp.tile([P, NPR, S], F8)              # packed head pairs
        Vg = kvp.tile([P, TP, H, 2, P], F8)         # [V|1/16|0...] slots

        xn_es = ExitStack()
        xnp = xn_es.enter_context(tc.tile_pool(name="xn", bufs=1))
        xnT = xnp.tile([P, KD, S], F8)              # feature-major LN1 out

        # ---------------- Phase 1: LN1 + transpose (+ V ds=0) ----------------
        wv_t = p2w.tile([P, KD, D], F8, bufs=1, tag="wv")
        nc.sync.dma_start(wv_t[:], Wv)

        def v_proj_t(ds, t):
            ps = prjps.tile([P, QS], F32, tag="pp")
            for kd in range(KD):
                nc.tensor.matmul(
                    ps[:], xnT[:, kd, t * P:(t + 1) * P],
                    wv_t[:, kd, ds * QS:(ds + 1) * QS],
                    start=(kd == 0), stop=(kd == KD - 1))
            dst = Vg[:, t // 2, ds * 8:(ds + 1) * 8, t % 2, 0:64]
            if bv_nz:
                nc.vector.tensor_tensor(
                    dst, ps[:].rearrange("p (h m) -> p h m", h=8),
                    bv_rep[:, ds * QS:(ds + 1) * QS].rearrange(
                        "p (h m) -> p h m", h=8), ALU.add)
            else:
                nc.vector.tensor_copy(
                    dst, ps[:].rearrange("p (h m) -> p h m", h=8))

        with tc.tile_pool(name="p1t", bufs=3) as p1t, \
             tc.tile_pool(name="p1s", bufs=4) as p1s, \
             tc.tile_pool(name="p1ps", bufs=4, space="PSUM") as p1ps:
            for t in range(ST):
                # interleaved zero-init of the V slot padding
                nc.gpsimd.memset(Vg[:, t // 2, :, t % 2, 64:65], 1.0 / 16.0)
                if t % 2 == 0:
                    nc.vector.memset(Vg[:, t // 2, :, t % 2, 65:128], 0.0)
                else:
                    nc.gpsimd.memset(Vg[:, t // 2, :, t % 2, 65:128], 0.0)
                x_t = p1t.tile([P, D], F32, tag="x_t")
                nc.sync.dma_start(x_t[:], xkv[t * P:(t + 1) * P, :])
                stats = p1s.tile([P, 2, 6], F32, tag="stats")
                xv = x_t[:].rearrange("p (s f) -> p s f", s=2)
                for s in range(2):
                    nc.vector.bn_stats(stats[:, s, :], xv[:, s, :])
                mv = p1s.tile([P, 2], F32, tag="mv")
                nc.vector.bn_aggr(mv[:], stats[:])
                std = p1s.tile([P, 1], F32, tag="std")
                nc.scalar.activation(std[:], mv[:, 1:2], AF.Sqrt, bias=eps_t[:])
                nc.vector.reciprocal(std[:], std[:])
                xn_t = p1t.tile([P, D], BF16, tag="xn_t")
                if affine:
                    xf = p1t.tile([P, D], F32, tag="xf")
                    nc.vector.tensor_scalar(
                        xf[:], x_t[:], scalar1=mv[:, 0:1], scalar2=std[:],
                        op0=ALU.subtract, op1=ALU.mult)
                    nc.vector.tensor_tensor(xf[:], xf[:], g1_rep[:], ALU.mult)
                    nc.vector.tensor_tensor(xn_t[:], xf[:], be1_rep[:], ALU.add)
                else:
                    nc.gpsimd.tensor_scalar(
                        xn_t[:], x_t[:], scalar1=mv[:, 0:1], scalar2=std[:],
                        op0=ALU.subtract, op1=ALU.mult)
                for j2 in range(KD // 2):
                    pst = p1ps.tile([P, 2, P], BF16, tag="tp1")
                    for hh in range(2):
                        nc.tensor.transpose(
                            pst[:, hh, :],
                            xn_t[:, (2 * j2 + hh) * P:(2 * j2 + hh + 1) * P],
                            ident[:])
                    if j2 % 2 == 0:
                        nc.vector.tensor_copy(
                            xnT[:, 2 * j2:2 * j2 + 2, t * P:(t + 1) * P], pst[:])
                    else:
                        nc.scalar.activation(
                            xnT[:, 2 * j2:2 * j2 + 2, t * P:(t + 1) * P],
                            pst[:], AF.Copy)
                v_proj_t(0, t)

        # attention psum pools open only after P1's transpose pool closed
        sc_es = ExitStack()
        scps = sc_es.enter_context(tc.tile_pool(name="scps", bufs=2, space="PSUM"))
        avps = sc_es.enter_context(tc.tile_pool(name="avps", bufs=1, space="PSUM"))

        qk_es = ExitStack()
        qkw = qk_es.enter_context(tc.tile_pool(name="qkw", bufs=2))
        wq_all = qkw.tile([P, KD, 4, 2, P], F8, bufs=1, tag="wqall")
        nc.sync.dma_start(wq_all[:], Wq.rearrange("d p a j m -> p d a j m"))

        def qk_proj(pr):
            wk_t = qkw.tile([P, 4, 2, P], F8, tag="wk")
            nc.sync.dma_start(wk_t[:], Wk[pr])
            for q in range(NQS):
                ps = prjps.tile([P, QS], F32, tag="pp")
                for dp in range(4):
                    nc.tensor.matmul(
                        ps[:], wq_all[:, pr, dp, :, :],
                        xnT[:, 2 * dp:2 * dp + 2, q * QS:(q + 1) * QS],
                        start=(dp == 0), stop=(dp == 3), perf_mode=PM.DoubleRow)
                qsl = slice(q * QS, (q + 1) * QS)
                nc.vector.tensor_scalar(
                    Qpad[:, 2 * pr, qsl], ps[:],
                    scalar1=maskA[:], scalar2=bqA[:, pr:pr + 1],
                    op0=ALU.mult, op1=ALU.add)
                nc.vector.tensor_scalar(
                    Qpad[:, 2 * pr + 1, qsl], ps[:],
                    scalar1=maskB[:], scalar2=bqB[:, pr:pr + 1],
                    op0=ALU.mult, op1=ALU.add)
            for q in range(NKS):
                ps = prjps.tile([P, QS], F32, tag="pp")
                for dp in range(4):
                    nc.tensor.matmul(
                        ps[:], wk_t[:, dp, :, :],
                        xnT[:, 2 * dp:2 * dp + 2, q * QS:(q + 1) * QS],
                        start=(dp == 0), stop=(dp == 3), perf_mode=PM.DoubleRow)
                nc.vector.tensor_scalar(
                    KT[:, pr, q * QS:(q + 1) * QS], ps[:],
                    scalar1=bk_t[:, pr:pr + 1], scalar2=None, op0=ALU.add)

        # ---------------- attention ----------------
        CH = [(0, 2), (2, 4), (4, 6), (6, 8), (8, 10), (10, 12), (12, 14), (14, 16)]
        # Schraudolph exp-to-fp8-bits constants (DVE/Pool offload of exp):
        # fp8 bitpattern of e^(ps/4096 - 3) ~= ps*SCA + SCB, clamped at 0.
        LOG2E = float(np.log2(np.e))
        SCA = 8.0 * LOG2E / 4096.0
        SCB = 8.0 * (7.0 - 3.0 * LOG2E) - 0.35
        U8 = mybir.dt.uint8
        # unit index (0..31) -> engine for its exp: ~7 DVE, 3 Pool, rest ACT
        DVE_UNITS = {2, 6, 10, 18, 22, 26, 30}
        POOL_UNITS = {14, 21, 29}

        # per-unit chunk routing: ACT keeps 5 of 8, DVE 2, Pool 1.
        CH_ENG = ["act", "act", "dve", "act", "act", "dve", "act", "dve"]

        def attn_scores(pr, hh, q, uidx):
            h = 2 * pr + hh
            qsl = slice(q * QS, (q + 1) * QS)
            e_s = epool.tile([P, ST, QS], F8, tag="e_s")
            for ci, (c0, c1) in enumerate(CH):
                ps_s = scps.tile([P, 2, QS], F32, tag="sc")
                for kt in range(c0, c1):
                    nc.tensor.matmul(
                        ps_s[:, kt - c0, :],
                        _dup2(KT[:, pr, kt * P:(kt + 1) * P]),
                        _dup2(Qpad[:, h, qsl]),
                        start=True, stop=True, perf_mode=PM.DoubleRow)
                eng = CH_ENG[ci]
                if eng == "act":
                    nc.scalar.activation(
                        e_s[:, c0:c1, :], ps_s[:, 0:c1 - c0, :], AF.Exp,
                        scale=1.0 / 4096.0, bias=neg3_t[:])
                else:
                    ve = nc.vector if eng == "dve" else nc.gpsimd
                    ve.tensor_scalar(
                        e_s[:, c0:c1, :].bitcast(U8), ps_s[:], scalar1=SCA,
                        scalar2=SCB, op0=ALU.mult, op1=ALU.add)
            return e_s

        def attn_av(pr, hh, q, e_s):
            h = 2 * pr + hh
            ps_av = avps.tile([P, QS], F32, tag="av")
            for tp in range(TP):
                nc.tensor.matmul(
                    ps_av[:], Vg[:, tp, h, :, :], e_s[:, 2 * tp:2 * tp + 2, :],
                    start=(tp == 0), stop=(tp == TP - 1), perf_mode=PM.DoubleRow)
            qsl = slice(q * QS, (q + 1) * QS)
            avc = avpool.tile([65, QS], F32, tag="avc")
            nc.scalar.activation(avc[:], ps_av[0:65, :], AF.Identity)
            nc.vector.reciprocal(avc[64:65, :], avc[64:65, :])
            rcd = dramp.tile([1, QS], F32, tag="rcd", bufs=2)
            nc.sync.dma_start(rcd[:], avc[64:65, :])
            bc = avpool.tile([64, QS], F32, tag="bc")
            nc.sync.dma_start(bc[:], _bcast_ap(rcd[0:1, :], 64))
            if hh == 0:
                nc.gpsimd.tensor_tensor(
                    RT[0:64, pr, qsl], avc[0:64, :], bc[:], ALU.mult)
            else:
                stB = avpool.tile([64, QS], F8, tag="stB")
                nc.gpsimd.tensor_tensor(stB[:], avc[0:64, :], bc[:], ALU.mult)
                nc.sync.dma_start(RT[64:128, pr, qsl], stB[:])

        def post_attn_a(q, prj, tpp):
            """O-proj + residual (in-place into x2) + LN2 -> xn2T for slice q.
            ACT work here is Identity-only, so it may interleave with exp."""
            qsl = slice(q * QS, (q + 1) * QS)
            attnT = attp.tile([P, KD, QS], BF16, tag="attnT")
            for dt in range(KD):
                ps = prj.tile([P, QS], F32, tag="pp")
                for dp in range(4):
                    nc.tensor.matmul(
                        ps[:], wo_all[:, dt, dp, :, :],
                        RT[:, 2 * dp:2 * dp + 2, qsl],
                        start=(dp == 0), stop=(dp == 3), perf_mode=PM.DoubleRow)
                nc.scalar.activation(
                    attnT[:, dt, :], ps[:], AF.Identity, scale=1.0 / 4096.0,
                    bias=bo_t[:, dt:dt + 1])
            for j in range(QS // P):
                tt = q * (QS // P) + j
                xr_t = p4t.tile([P, D], F32, tag="xr_t")
                nc.sync.dma_start(xr_t[:], xkv[tt * P:(tt + 1) * P, :])
                for m2 in range(KD // 2):
                    pst = tpp.tile([P, 2, P], BF16, tag="tp", bufs=1)
                    for hh in range(2):
                        nc.tensor.transpose(
                            pst[:, hh, :],
                            attnT[:, 2 * m2 + hh, j * P:(j + 1) * P], ident[:])
                    nc.vector.tensor_tensor(
                        x2[:, tt, 2 * m2 * P:(2 * m2 + 2) * P],
                        pst[:].rearrange("p a m -> p (a m)"),
                        xr_t[:, 2 * m2 * P:(2 * m2 + 2) * P], ALU.add)

        def mlp(q, pap, p5t, prj, tpp):
            """LN2 + Gelu-MLP + final residual (in place into x2) + out DMA.
            Emitted after the exp stream ends (Gelu/Sqrt force ACT table swaps)."""
            xn2T = pap.tile([P, KD, QS], MD, tag="xn2T")
            for j in range(QS // P):
                tt = q * (QS // P) + j
                stats = p4s.tile([P, 2, 6], F32, tag="st2")
                x2v = x2[:, tt, :].rearrange("p (s f) -> p s f", s=2)
                for s in range(2):
                    nc.vector.bn_stats(stats[:, s, :], x2v[:, s, :])
                mv = p4s.tile([P, 2], F32, tag="mv2")
                nc.vector.bn_aggr(mv[:], stats[:])
                std = p4s.tile([P, 1], F32, tag="std2")
                nc.scalar.activation(std[:], mv[:, 1:2], AF.Sqrt, bias=eps_t[:])
                nc.vector.reciprocal(std[:], std[:])
                xn2_t = p4t.tile([P, D], BF16, tag="xn2_t")
                if affine:
                    xf2 = p4t.tile([P, D], F32, tag="xf2")
                    nc.vector.tensor_scalar(
                        xf2[:], x2[:, tt, :], scalar1=mv[:, 0:1],
                        scalar2=std[:], op0=ALU.subtract, op1=ALU.mult)
                    nc.vector.tensor_tensor(xf2[:], xf2[:], g2_rep[:], ALU.mult)
                    nc.vector.tensor_tensor(xn2_t[:], xf2[:], be2_rep[:], ALU.add)
                else:
                    nc.gpsimd.tensor_scalar(
                        xn2_t[:], x2[:, tt, :], scalar1=mv[:, 0:1],
                        scalar2=std[:], op0=ALU.subtract, op1=ALU.mult)
                for j2 in range(KD // 2):
                    pst = tpp.tile([P, 2, P], BF16, tag="tp", bufs=1)
                    for hh in range(2):
                        nc.tensor.transpose(
                            pst[:, hh, :],
                            xn2_t[:, (2 * j2 + hh) * P:(2 * j2 + hh + 1) * P],
                            ident[:])
                    nc.vector.tensor_copy(
                        xn2T[:, 2 * j2:2 * j2 + 2, j * P:(j + 1) * P], pst[:])
            sc1 = 1.0 / 32.0 if mlp_mode == "f8" else 1.0
            sc2 = 1.0 if mlp_mode == "bf" else 1.0 / 64.0
            h1T = pap.tile([P, FT, QS], HD, tag="h1T")
            for fc in range(FT // 4):
                if mlp_mode == "f8":
                    w1_t = p5t.tile([P, 4, 4, 2, P], F8, tag="w1")
                    nc.sync.dma_start(
                        w1_t[:], W1[4 * fc:4 * fc + 4].rearrange(
                            "f p a j m -> p f a j m"))
                else:
                    w1_t = p5t.tile([P, 4, KD, P], BF16, tag="w1")
                    nc.sync.dma_start(
                        w1_t[:], W1[4 * fc:4 * fc + 4].rearrange(
                            "f p k m -> p f k m"))
                for fi in range(4):
                    ft = 4 * fc + fi
                    ps = prj.tile([P, QS], F32, tag="pp")
                    if mlp_mode == "f8":
                        for dp in range(4):
                            nc.tensor.matmul(
                                ps[:], w1_t[:, fi, dp, :, :],
                                xn2T[:, 2 * dp:2 * dp + 2, :],
                                start=(dp == 0), stop=(dp == 3),
                                perf_mode=PM.DoubleRow)
                    else:
                        for kd in range(KD):
                            nc.tensor.matmul(
                                ps[:], w1_t[:, fi, kd, :], xn2T[:, kd, :],
                                start=(kd == 0), stop=(kd == KD - 1))
                    nc.scalar.activation(h1T[:, ft, :], ps[:], AF.Gelu,
                                         scale=sc1, bias=b1_t[:, ft:ft + 1])

            def h2_epilogue(dt, ps):
                outT = p5t.tile([P, QS], BF16, tag="outT")
                nc.scalar.activation(outT[:], ps[:], AF.Identity, scale=sc2,
                                     bias=b2_t[:, dt:dt + 1])
                for j in range(QS // P):
                    tt = q * (QS // P) + j
                    pst = tpp.tile([P, 2, P], BF16, tag="tp", bufs=1)
                    nc.tensor.transpose(
                        pst[:, 0, :], outT[:, j * P:(j + 1) * P], ident[:])
                    nc.vector.tensor_tensor(
                        x2[:, tt, dt * P:(dt + 1) * P], pst[:, 0, :],
                        x2[:, tt, dt * P:(dt + 1) * P], ALU.add)

            prev5 = None
            for dt in range(KD):
                if mlp_mode == "bf":
                    w2_t = p5t.tile([P, FT, P], BF16, tag="w2")
                else:
                    w2_t = p5t.tile([P, 16, 2, P], F8, tag="w2")
                nc.sync.dma_start(w2_t[:], W2[dt])
                ps = prj.tile([P, QS], F32, tag="p2b")
                if mlp_mode == "bf":
                    for ft in range(FT):
                        nc.tensor.matmul(
                            ps[:], w2_t[:, ft, :], h1T[:, ft, :],
                            start=(ft == 0), stop=(ft == FT - 1))
                else:
                    for fp in range(16):
                        nc.tensor.matmul(
                            ps[:], w2_t[:, fp, :, :],
                            h1T[:, 2 * fp:2 * fp + 2, :],
                            start=(fp == 0), stop=(fp == 15),
                            perf_mode=PM.DoubleRow)
                if prev5 is not None:
                    h2_epilogue(*prev5)
                prev5 = (dt, ps)
            h2_epilogue(*prev5)
            for j in range(QS // P):
                tt = q * (QS // P) + j
                nc.sync.dma_start(out[tt * P:(tt + 1) * P, :], x2[:, tt, :])

        # ---- driver ----
        wo_all = p2w.tile([P, KD, 4, 2, P], F8, bufs=1, tag="wo")
        nc.sync.dma_start(wo_all[:], Wo.rearrange("d p a j m -> p d a j m"))
        prev_u = None
        for pr in range(NPR):
            qk_proj(pr)
            for hh in range(2):
                e_s = attn_scores(pr, hh, 0, 2 * pr + hh)
                if prev_u is not None:
                    attn_av(*prev_u)
                prev_u = (pr, hh, 0, e_s)
            if pr == 3:
                for t in range(ST):
                    v_proj_t(1, t)
        attn_av(*prev_u)
        prev_u = None
        qk_es.close()
        xn_es.close()

        for pr in range(NPR):
            for hh in range(2):
                e_s = attn_scores(pr, hh, 1, 16 + 2 * pr + hh)
                if prev_u is not None:
                    attn_av(*prev_u)
                prev_u = (pr, hh, 1, e_s)
            if pr == 0:
                post_attn_a(0, prjps, tpps)
        attn_av(*prev_u)
        kv_es.close()

        # attention psum pools are done; reopen with deeper buffering
        sc_es.close()
        tail_es = ExitStack()
        tailps = tail_es.enter_context(
            tc.tile_pool(name="tailps", bufs=2, space="PSUM"))
        pap = tail_es.enter_context(tc.tile_pool(name="pa", bufs=1))
        p5tt = tail_es.enter_context(tc.tile_pool(name="p5tl", bufs=2))
        post_attn_a(1, tailps, tailps)
        mlp(0, pap, p5tt, tailps, tailps)
        mlp(1, pap, p5tt, tailps, tailps)
        tail_es.close()

        ps_es.close()
        w_es.close()
        rt_es.close()
        x2_es.close()
        es.close()

    nc.compile()
    return nc


def kernel(**inputs):
    import ml_dtypes
    F8NP = ml_dtypes.float8_e4m3
    BFNP = ml_dtypes.bfloat16

    inputs = {k: np.ascontiguousarray(np.asarray(v), dtype=np.float32)
              for k, v in inputs.items()}
    affine = not (
        np.all(inputs["ln1_g"] == 1.0) and np.all(inputs["ln1_b"] == 0.0)
        and np.all(inputs["ln2_g"] == 1.0) and np.all(inputs["ln2_b"] == 0.0))
    bv_nz = bool(np.any(inputs["bv"] != 0.0))
    key = ("nc", affine, bv_nz, MLP_MODE)
    if key not in _CACHE:
        _CACHE[key] = _build(affine=affine, bv_nz=bv_nz, mlp_mode=MLP_MODE)
    nc = _CACHE[key]

    def tile_dr(W, sc):
        # [Din, Dout] -> [Dout/128, 128, Din/256, 2, 128] fp8 (x sc)
        Din, Dout = W.shape
        return np.ascontiguousarray(
            (W * sc).reshape(Din // 256, 2, P, Dout // P, P)
            .transpose(3, 2, 0, 1, 4)).astype(F8NP)

    def tile_bf(W, n_out):
        Din, Dout = W.shape
        return np.ascontiguousarray(
            W.reshape(Din // P, P, n_out, P).transpose(2, 1, 0, 3)).astype(BFNP)

    x = inputs["x"]
    shared = {
        "Wq": tile_dr(inputs["Wq"], 16.0), "Wk": tile_dr(inputs["Wk"], 16.0),
        "Wo": tile_dr(inputs["Wo"], 16.0),
        # Wv moving: [128, 8 kd, 1024] fp8 x16
        "Wv": np.ascontiguousarray(
            (inputs["Wv"] * 16.0).reshape(KD, P, D).transpose(1, 0, 2)
        ).astype(F8NP),
        "W1": (tile_dr(inputs["W1"], 32.0) if MLP_MODE == "f8"
               else tile_bf(inputs["W1"], FT)),
        "W2": (tile_bf(inputs["W2"], KD) if MLP_MODE == "bf"
               else tile_dr(inputs["W2"], 64.0)),
        "bq": inputs["bq"] * 16.0, "bk": inputs["bk"] * 16.0,
        "bv": inputs["bv"] * 16.0, "bo": inputs["bo"],
        "b1": inputs["b1"], "b2": inputs["b2"],
        "g1": inputs["ln1_g"], "be1": inputs["ln1_b"],
        "g2": inputs["ln2_g"], "be2": inputs["ln2_b"],
    }
    in_maps = []
    for c in range(8):
        b, half = c // 2, c % 2
        m = dict(shared)
        m["xkv"] = np.ascontiguousarray(
            np.concatenate([x[b, half * NQ:(half + 1) * NQ, :],
                            x[b, (1 - half) * NQ:(2 - half) * NQ, :]], axis=0))
        in_maps.append(m)

    trace = bool(int(os.environ.get("KERNEL_TRACE", "0")))
    kw = {}
    if trace:
        kw = dict(trace=True, tmpdir=os.environ.get("KERNEL_TRACE_DIR") or None)
    res = bass_utils.run_bass_kernel_spmd(nc, in_maps, core_ids=list(range(8)), **kw)
    _CACHE["last_results"] = res
    _CACHE["nc"] = nc
    _CACHE["last_in_maps"] = in_maps

    outa = np.empty((B, S, D), dtype=np.float32)
    for c in range(8):
        b, half = c // 2, c % 2
        outa[b, half * NQ:(half + 1) * NQ, :] = res.results[c]["out"]
    return outa
